# revision 1
# baseline (speedup 1.0000x reference)
"""Multi-head causal attention (B=2, T=2048, E=1024, H=16, D=64) on 8 TRN2
NeuronCores, tensor-parallel over heads (2 heads per core).

All matmul inputs are fp32r (full-rate reduced-precision fp32 on the PE,
~1.8e-4 rel err); PSUM accumulation is fp32. Dataflow per core:
  host:  xT = x^T [E, B*T] (same on all cores); wqkv_c [E, 384] (this
         core's 2 heads of Wq|Wk|Wv); wproj_c = Wproj[128c:128c+128, :]
  1. qT[d,t], kT[d,t], vT[d,t] = wqkv_c^T @ xT    (PSUM accum over E tiles)
     v[s,d] tiles via PE transpose of vT, + a ones column per head used to
     produce the softmax denominator inside the av matmul
  2. per (b, 512-t-block), heads interleaved per 128-s-tile:
       weiT[s,t] = kT^T q       (scores computed directly transposed, K=64)
       Exp on ACT (scale=E^-0.5) PSUM->SBUF; causal mask = multiplicative
       0/1 tril on the diagonal 128-chunk + zero-fill of below-diag chunks
       avT_aug[65,t] += [v_h|1]^T @ expweiT  (row 64 = sum_s = denominator)
     then one batched DVE reciprocal (both heads at partitions 0/32) and a
     K=33 PE matmul broadcasts 1/l across partitions; normalize fuses with
     the PSUM->SBUF copy into the merged-head proj lhsT avT_sb [128, t].
  3. y_partial[t,e] = avT_sb^T @ wproj_c per 128-t-chunk; DMA out.
  host:  y = sum_c y_partial_c + bproj  (tensor-parallel partial sums).

Infra notes: this container's walrus accepts at most ONE semaphore wait
per instruction (_split_multi_waits hoists extras onto EventSemaphores),
and custom-DVE ops / gpsimd partition_broadcast do not compile ("ISA
wrong length"). The ACT engine must run a single function (Exp) — mixing
in Copy/Reciprocal thrashes the activation table at ~1.3us per swap.
"""
import sys
import types

import numpy as np

B, T, E, H, D = 2, 2048, 1024, 16, 64
N_CORES = 8
HPC = H // N_CORES          # heads per core = 2
BT = B * T                  # 4096
DPC = HPC * D               # 128 head-dims per core
SCALE = 1.0 / float(np.sqrt(E))  # NOTE: reference scales by E**-0.5
NEG = -1e9


def _install_ntff_hook():
    if 'antenv.axon_hooks' in sys.modules:
        return
    try:
        sys.path.insert(0, '/root/.axon_site')
        from trn_agent_boot.trn_boot import _ntff_profile_via_ctypes
        hook = _ntff_profile_via_ctypes('/opt/axon/libaxon_pjrt.so')
        mod = types.ModuleType('antenv.axon_hooks')
        mod.get_axon_ntff_profile_hook = lambda: hook
        mod.set_axon_ntff_profile_hook = lambda h: None
        sys.modules['antenv.axon_hooks'] = mod
    except Exception:
        pass


def _split_multi_waits(nc, mybir):
    """This walrus build rejects >1 sync-wait per instruction. Hoist extra
    waits onto EventSemaphore instructions on the same engine just before."""
    for f in nc.m.functions:
        for bb in f.blocks:
            new_insts = []
            changed = False
            for inst in bb.instructions:
                si = inst.sync_info
                if si is not None and len(si.on_wait) > 1:
                    extra = list(si.on_wait[:-1])
                    keep = si.on_wait[-1]
                    for w in extra:
                        ev = mybir.InstEventSemaphore(
                            name=f"I-{nc.next_id()}", ins=[], outs=[])
                        ev.engine = inst.engine
                        ev.sync_info = mybir.SyncInfo(on_wait=[w], on_update=[])
                        new_insts.append(ev)
                    del si.on_wait[:]
                    si.on_wait.append(keep)
                    changed = True
                new_insts.append(inst)
            if changed:
                bb.instructions = new_insts


def _build_nc():
    import concourse.bass as bass
    import concourse.mybir as mybir
    import concourse.tile as tile
    from concourse.masks import make_identity

    f32 = mybir.dt.float32
    f32r = mybir.dt.float32r
    bf16 = mybir.dt.bfloat16
    EXP = mybir.ActivationFunctionType.Exp

    nc = bass.Bass('TRN2', num_devices=N_CORES)
    xt = nc.dram_tensor('xt', [E, BT], f32r, kind='ExternalInput')
    wqkv = nc.dram_tensor('wqkv', [E, 3 * DPC], f32r, kind='ExternalInput')
    wproj = nc.dram_tensor('wproj', [DPC, E], f32r, kind='ExternalInput')
    y = nc.dram_tensor('y', [BT, E], f32, kind='ExternalOutput')

    NTB1 = BT // 512            # 8 t-blocks in phase 1
    NE = E // 128               # 8 e-tiles
    NTB = T // 512              # 4 t-blocks per batch in phase 2
    NST = T // 128              # 16 s-tiles per batch

    with tile.TileContext(nc) as tc:
        with tc.tile_pool(name='consts', bufs=1) as consts, \
             tc.tile_pool(name='big', bufs=1) as big, \
             tc.tile_pool(name='work', bufs=1) as work, \
             tc.tile_pool(name='ps', bufs=1, space='PSUM') as ps:

            # ---- constants ----
            ident = consts.tile([128, 128], f32)
            make_identity(nc, ident)
            # additive mask for the diagonal chunk of weiT [s,t]:
            # keep (0) where t >= s, NEG where t < s
            tmask_f = consts.tile([128, 128], f32)
            nc.gpsimd.memset(tmask_f[:], 1.0)
            nc.gpsimd.affine_select(
                out=tmask_f[:], in_=tmask_f[:],
                compare_op=mybir.AluOpType.is_ge,
                fill=0.0, base=0, pattern=[[1, 128]], channel_multiplier=-1)
            tmask = consts.tile([128, 128], f32r)
            nc.vector.tensor_copy(tmask[:], tmask_f[:])
            # sel [33,128]: row0 -> partitions 0-63 (head0), row32 -> 64-127
            sel_f32 = consts.tile([33, 128], f32)
            nc.gpsimd.memset(sel_f32[:], 0.0)
            nc.gpsimd.memset(sel_f32[0:1, 0:64], 1.0)
            nc.gpsimd.memset(sel_f32[32:33, 64:128], 1.0)
            sel_bc = consts.tile([33, 128], f32r)   # lhsT of bcast matmul
            nc.vector.tensor_copy(sel_bc[:], sel_f32[:])

            # ---- weights ----
            wqkv_sb = [consts.tile([128, 3 * DPC], f32r, name=f'wqkv{k}')
                       for k in range(NE)]
            for k in range(NE):
                nc.sync.dma_start(out=wqkv_sb[k][:], in_=wqkv[k * 128:(k + 1) * 128, :])
            wproj_sb = consts.tile([DPC, E], f32r)
            nc.sync.dma_start(out=wproj_sb[:], in_=wproj[:])

            # ---- persistent activations ----
            qT_sb = [big.tile([128, 512], f32r, name=f'q{j}')
                     for j in range(NTB1)]
            kT_sb = [big.tile([128, 512], f32r, name=f'k{j}')
                     for j in range(NTB1)]
            # v tiles [s,d] per 128-s-tile, layout [128, 2, 65]: per head 64
            # dims + ones column (softmax denominator via matmul)
            v_sb = [big.tile([128, 2, 65], f32r, name=f'v{si}')
                    for si in range(2 * NST)]
            for si in range(2 * NST):
                nc.gpsimd.memset(v_sb[si][:, :, 64:65].bitcast(mybir.dt.uint32), 1065353216)

            # ---- phase 1: one 512-t-block of QKV projections ----
            def emit_qkv_block(tb):
                ts = tb * 512
                q_ps = ps.tile([128, 512], f32, tag='acc', bufs=3)
                k_ps = ps.tile([128, 512], f32, tag='acc', bufs=3)
                vt_ps = ps.tile([128, 512], f32, tag='acc', bufs=3)
                for k in range(NE):
                    xt_t = work.tile([128, 512], f32r, tag='xt', bufs=8)
                    nc.sync.dma_start(
                        out=xt_t[:], in_=xt[k * 128:(k + 1) * 128, ts:ts + 512])
                    st, sp = (k == 0), (k == NE - 1)
                    nc.tensor.matmul(q_ps[:], wqkv_sb[k][:, 0:128], xt_t[:],
                                     start=st, stop=sp)
                    nc.tensor.matmul(k_ps[:], wqkv_sb[k][:, 128:256], xt_t[:],
                                     start=st, stop=sp)
                    nc.tensor.matmul(vt_ps[:], wqkv_sb[k][:, 256:384], xt_t[:],
                                     start=st, stop=sp)
                nc.vector.tensor_copy(qT_sb[tb][:], q_ps[:])
                nc.vector.tensor_copy(kT_sb[tb][:], k_ps[:])
                vt_sb = work.tile([128, 512], f32, tag='vt', bufs=2)
                nc.vector.tensor_copy(vt_sb[:], vt_ps[:])
                # transpose vT -> v [s, d] per 128-chunk
                for sc in range(4):
                    si = tb * 4 + sc
                    vtr = ps.tile([128, 128], f32, tag='misc', bufs=2)
                    nc.tensor.transpose(vtr[:], vt_sb[:, sc * 128:(sc + 1) * 128],
                                        ident[:])
                    nc.vector.tensor_copy(
                        v_sb[si][:, :, 0:64],
                        vtr.rearrange('p (h e) -> p h e', h=2))

            # ---- phase 2+3: attention + projection for one t-block ----
            def emit_attn_block(b, tb):
                t0 = b * T + tb * 512          # global t offset
                n_si = 4 * (tb + 1)            # s-tiles (causal)
                avT_sb = work.tile([128, 512], f32r, tag='avT', bufs=3)
                lrow = work.tile([33, 512], f32, tag='lrow', bufs=2)
                nc.gpsimd.memset(lrow[:], 1.0)
                av_pss = [ps.tile([65, 512], f32, tag='acc', bufs=3,
                                  name=f'av{b}_{tb}_{h}')
                          for h in range(HPC)]
                for si in range(n_si):
                    s0 = b * T + si * 128
                    sblk, srem = divmod(s0 - b * T, 512)
                    sblk += b * NTB
                    woff = 0
                    if si >= 4 * tb:           # diagonal region
                        woff = (si - 4 * tb) * 128
                    for h in range(HPC):
                        hd = h * 64
                        w_ps = ps.tile([128, 512], f32, tag='wei', bufs=3)
                        nc.tensor.matmul(
                            w_ps[:],
                            kT_sb[sblk][hd:hd + 64, srem:srem + 128],
                            qT_sb[b * NTB + tb][hd:hd + 64, :],
                            start=True, stop=True)
                        wt = work.tile([128, 512], f32r, tag='weiT', bufs=28)
                        if woff > 0:
                            nc.gpsimd.memset(
                                wt[:, 0:woff].bitcast(mybir.dt.uint32), 0)
                        nc.scalar.activation(wt[:, woff:512], w_ps[:, woff:512],
                                             EXP, scale=SCALE)
                        if si >= 4 * tb:
                            nc.vector.tensor_mul(wt[:, woff:woff + 128],
                                                 wt[:, woff:woff + 128],
                                                 tmask[:])
                        nc.tensor.matmul(
                            av_pss[h][:], v_sb[b * NST + si][:, h, :], wt[:],
                            start=(si == 0), stop=(si == n_si - 1))
                for h in range(HPC):
                    # stash softmax denominator row (partition 0 / 32)
                    nc.vector.tensor_copy(lrow[32 * h:32 * h + 1, :],
                                          av_pss[h][64:65, :])
                rc2 = work.tile([33, 512], f32, tag='rc', bufs=2)
                nc.vector.reciprocal(rc2[:], lrow[:])
                rc2r = work.tile([33, 512], f32r, tag='rcr', bufs=2)
                nc.vector.tensor_copy(rc2r[:], rc2[:])
                bc_ps = ps.tile([128, 512], f32, tag='misc', bufs=2)
                nc.tensor.matmul(bc_ps[:], sel_bc[:], rc2r[:],
                                 start=True, stop=True)
                bc_sb = work.tile([128, 512], f32, tag='bcs', bufs=2)
                nc.vector.tensor_copy(bc_sb[:], bc_ps[:])
                for h in range(HPC):
                    hd = h * 64
                    nc.vector.tensor_mul(avT_sb[hd:hd + 64, :],
                                         av_pss[h][0:64, :],
                                         bc_sb[hd:hd + 64, :])
                # ---- projection for this 512-t-block ----
                for tc4 in range(4):
                    for eb in range(2):
                        y_ps = ps.tile([128, 512], f32, tag='misc', bufs=2)
                        nc.tensor.matmul(
                            y_ps[:],
                            avT_sb[:, tc4 * 128:(tc4 + 1) * 128],
                            wproj_sb[:, eb * 512:(eb + 1) * 512],
                            start=True, stop=True)
                        y_sb = work.tile([128, 512], f32, tag='ysb', bufs=6)
                        nc.vector.tensor_copy(y_sb[:], y_ps[:])
                        nc.sync.dma_start(
                            out=y[t0 + tc4 * 128:t0 + (tc4 + 1) * 128,
                                  eb * 512:(eb + 1) * 512],
                            in_=y_sb[:])

            for tb in range(NTB1):
                emit_qkv_block(tb)
            for b in range(B):
                for tb in range(NTB):
                    emit_attn_block(b, tb)

    import concourse.mybir as mybir2
    _split_multi_waits(nc, mybir2)
    return nc


_CACHE = {}


def kernel(x, Wq, Wk, Wv, Wproj, bproj):
    _install_ntff_hook()
    from concourse.bass_utils import run_bass_kernel_spmd

    x = np.asarray(x, dtype=np.float32)
    Wq = np.asarray(Wq, dtype=np.float32)
    Wk = np.asarray(Wk, dtype=np.float32)
    Wv = np.asarray(Wv, dtype=np.float32)
    Wproj = np.asarray(Wproj, dtype=np.float32)
    bproj = np.asarray(bproj, dtype=np.float32)

    if 'nc' not in _CACHE:
        _CACHE['nc'] = _build_nc()
    nc = _CACHE['nc']

    xT = np.ascontiguousarray(x.reshape(BT, E).T)
    in_maps = []
    for c in range(N_CORES):
        h0 = HPC * c
        wqkv_c = np.concatenate(
            [Wq[h0], Wq[h0 + 1], Wk[h0], Wk[h0 + 1], Wv[h0], Wv[h0 + 1]],
            axis=1)                                         # [E, 384]
        wproj_c = np.ascontiguousarray(Wproj[DPC * c: DPC * (c + 1)])
        in_maps.append({'xt': xT, 'wqkv': np.ascontiguousarray(wqkv_c),
                        'wproj': wproj_c})

    res = run_bass_kernel_spmd(nc, in_maps, list(range(N_CORES)))
    ysum = np.zeros((BT, E), dtype=np.float64)
    for c in range(N_CORES):
        ysum += res.results[c]['y'].astype(np.float64)
    out = (ysum + bproj.astype(np.float64)).astype(np.float32)
    return out.reshape(B, T, E)



# revision 4
# speedup vs baseline: 1.1073x; 1.1073x over previous
"""Multi-head causal attention (B=2, T=2048, E=1024, H=16, D=64) on 8 TRN2
NeuronCores, tensor-parallel over heads (2 heads per core).

v2 dataflow per core (vs v1: +fp8 DoubleRow QKV, bf16 AV path, causal
matmul trimming, software-pipelined normalize/proj, ACT-assisted y copies):
  host:  xt8  [4,128,2,BT] fp8e4 = x^T with e=(c,i,p) pair layout for
         DoubleRow (K_eff=256); xtb [E,BT] bf16; wqk8 [4,128,2,256] fp8
         (this core's 2 heads of Wq|Wk, pre-scaled x16); wvb [E,128] bf16;
         wproj [128,E] f32r.
  1. qT/kT [128,512] f32 PSUM via 4 fp8-DoubleRow matmuls each (2x PE rate),
     cast to f32r SBUF; vT via 8 bf16 matmuls, cast bf16, PE-transposed
     (bf16 transpose -> bf16 PSUM) into v[s,d] tiles + ones column (softmax
     denominator via the AV matmul row 64).
  2. per (b, 512-t-block): per (si, h): weiT = kT^T q (f32r, N trimmed to
     [woff(-ish):512] on diagonal tiles); Exp on ACT (scale=E^-0.5/256)
     PSUM->SBUF bf16 [woff:512]; diagonal 0/1-tril mask multiplied on
     GPSIMD; avT_aug[65, woff:512] += [v|1]^T @ wt (bf16).  Normalize+proj
     of block i runs pipelined behind attention of block i+1: denominator
     row -> reciprocal (DVE) -> K=33 PE broadcast -> per-head multiply into
     avT f32r; proj y = avT^T @ wproj; y tiles copied PSUM->SBUF bf16
     alternating ACT(Copy, same act table as Exp)/DVE, DMA out bf16.
  3. batch-1 QKV is emitted between attn(0,3) and normproj(0,3) so PE never
     idles and ACT pipeline fill stays ~1 batch of QKV.
  host:  y = sum_c y_partial_c + bproj.

Infra notes: walrus accepts one semaphore wait per instruction
(_split_multi_waits); gpsimd cannot touch PSUM; DVE/Pool have no divide;
DVE reciprocal is ~4us per op regardless of shape; fp8 DoubleRow needs
[p,2,*] operand layout and is only profitable at K_eff=256 (K=64 form is
LDW-bound); partial-N f32r matmuls drop to 1/4 rate below N=256 so
diagonal trims clamp at N>=256; bf16 affine_select miscompiles (masks via
f32 affine_select + cast); Copy/Identity/Ln share the Exp ACT table set.
"""
import sys
import types

import numpy as np
import ml_dtypes

B, T, E, H, D = 2, 2048, 1024, 16, 64
N_CORES = 8
HPC = H // N_CORES          # heads per core = 2
BT = B * T                  # 4096
DPC = HPC * D               # 128 head-dims per core
WSCALE = 16.0               # host pre-scale on Wq/Wk before fp8 quant
SCALE = 1.0 / float(np.sqrt(E))
EXP_SCALE = SCALE / (WSCALE * WSCALE)


def _install_ntff_hook():
    if 'antenv.axon_hooks' in sys.modules:
        return
    try:
        sys.path.insert(0, '/root/.axon_site')
        from trn_agent_boot.trn_boot import _ntff_profile_via_ctypes
        hook = _ntff_profile_via_ctypes('/opt/axon/libaxon_pjrt.so')
        mod = types.ModuleType('antenv.axon_hooks')
        mod.get_axon_ntff_profile_hook = lambda: hook
        mod.set_axon_ntff_profile_hook = lambda h: None
        sys.modules['antenv.axon_hooks'] = mod
    except Exception:
        pass


def _split_multi_waits(nc, mybir):
    """This walrus build rejects >1 sync-wait per instruction. Hoist extra
    waits onto EventSemaphore instructions on the same engine just before."""
    for f in nc.m.functions:
        for bb in f.blocks:
            new_insts = []
            changed = False
            for inst in bb.instructions:
                si = inst.sync_info
                if si is not None and len(si.on_wait) > 1:
                    extra = list(si.on_wait[:-1])
                    keep = si.on_wait[-1]
                    for w in extra:
                        ev = mybir.InstEventSemaphore(
                            name=f"I-{nc.next_id()}", ins=[], outs=[])
                        ev.engine = inst.engine
                        ev.sync_info = mybir.SyncInfo(on_wait=[w], on_update=[])
                        new_insts.append(ev)
                    del si.on_wait[:]
                    si.on_wait.append(keep)
                    changed = True
                new_insts.append(inst)
            if changed:
                bb.instructions = new_insts


def _build_nc():
    import concourse.bass as bass
    import concourse.mybir as mybir
    import concourse.tile as tile
    from concourse.masks import make_identity

    f32 = mybir.dt.float32
    f32r = mybir.dt.float32r
    bf16 = mybir.dt.bfloat16
    f8 = mybir.dt.float8e4
    EXP = mybir.ActivationFunctionType.Exp
    CPY = mybir.ActivationFunctionType.Copy
    DR = mybir.MatmulPerfMode.DoubleRow

    nc = bass.Bass('TRN2', num_devices=N_CORES)
    xt8 = nc.dram_tensor('xt8', [4, 128, 2, BT], f8, kind='ExternalInput')
    xtb = nc.dram_tensor('xtb', [E, BT], bf16, kind='ExternalInput')
    wqk8 = nc.dram_tensor('wqk8', [4, 128, 2, 256], f8, kind='ExternalInput')
    wvb = nc.dram_tensor('wvb', [E, DPC], bf16, kind='ExternalInput')
    wproj = nc.dram_tensor('wproj', [DPC, E], f32r, kind='ExternalInput')
    y = nc.dram_tensor('y', [BT, E], bf16, kind='ExternalOutput')

    NTB1 = BT // 512            # 8 t-blocks in phase 1
    NE = E // 128               # 8 e-tiles
    NTB = T // 512              # 4 t-blocks per batch in phase 2
    NST = T // 128              # 16 s-tiles per batch

    with tile.TileContext(nc) as tc:
        with tc.tile_pool(name='consts', bufs=1) as consts, \
             tc.tile_pool(name='big', bufs=1) as big, \
             tc.tile_pool(name='work', bufs=1) as work, \
             tc.tile_pool(name='ps', bufs=1, space='PSUM') as ps:

            # ---- constants ----
            ident = consts.tile([128, 128], f32)
            make_identity(nc, ident)
            identb = consts.tile([128, 128], bf16)
            nc.vector.tensor_copy(identb[:], ident[:])
            # multiplicative 0/1 tril mask (keep where t >= s) for the
            # diagonal 128-chunk, bf16 (affine_select only works in f32)
            tmask_f = consts.tile([128, 128], f32)
            nc.gpsimd.memset(tmask_f[:], 1.0)
            nc.gpsimd.affine_select(
                out=tmask_f[:], in_=tmask_f[:],
                compare_op=mybir.AluOpType.is_ge,
                fill=0.0, base=0, pattern=[[1, 128]], channel_multiplier=-1)
            tmask_b = consts.tile([128, 128], bf16)
            nc.vector.tensor_copy(tmask_b[:], tmask_f[:])
            # sel [33,128]: row0 -> partitions 0-63 (head0), row32 -> 64-127
            sel_f32 = consts.tile([33, 128], f32)
            nc.gpsimd.memset(sel_f32[:], 0.0)
            nc.gpsimd.memset(sel_f32[0:1, 0:64], 1.0)
            nc.gpsimd.memset(sel_f32[32:33, 64:128], 1.0)
            sel_bc = consts.tile([33, 128], f32r)
            nc.vector.tensor_copy(sel_bc[:], sel_f32[:])

            # ---- weights ----
            wqk8_sb = [consts.tile([128, 2, 256], f8, name=f'wqk{c}')
                       for c in range(4)]
            for c in range(4):
                nc.sync.dma_start(out=wqk8_sb[c][:], in_=wqk8[c])
            wvb_sb = [consts.tile([128, DPC], bf16, name=f'wv{k}')
                      for k in range(NE)]
            for k in range(NE):
                nc.sync.dma_start(out=wvb_sb[k][:],
                                  in_=wvb[k * 128:(k + 1) * 128, :])
            wproj_sb = consts.tile([DPC, E], f32r)
            nc.sync.dma_start(out=wproj_sb[:], in_=wproj[:])

            # ---- persistent activations ----
            qT_sb = [big.tile([128, 512], f32r, name=f'q{j}')
                     for j in range(NTB1)]
            kT_sb = [big.tile([128, 512], f32r, name=f'k{j}')
                     for j in range(NTB1)]
            # v tiles [s, (h, d)] bf16 per 128-s-tile; col 64 per head = ones
            v_sb = [big.tile([128, 2, 65], bf16, name=f'v{si}')
                    for si in range(2 * NST)]
            for si in range(2 * NST):
                nc.gpsimd.memset(v_sb[si][:, :, 64:65], 1.0)

            # ---- phase 1: one 512-t-block of QKV projections ----
            def emit_qkv_block(tb):
                ts = tb * 512
                q_ps = ps.tile([128, 512], f32, tag='acc', bufs=4)
                k_ps = ps.tile([128, 512], f32, tag='acc', bufs=4)
                vt_ps = ps.tile([128, 512], f32, tag='misc', bufs=2)
                # v first: its PSUM->SBUF cast then overlaps the q/k matmuls,
                # so the PE transposes below don't stall on the cast
                for k in range(NE):
                    xb_t = work.tile([128, 512], bf16, tag='xb', bufs=8)
                    nc.sync.dma_start(
                        out=xb_t[:], in_=xtb[k * 128:(k + 1) * 128, ts:ts + 512])
                    nc.tensor.matmul(vt_ps[:], wvb_sb[k][:], xb_t[:],
                                     start=(k == 0), stop=(k == NE - 1))
                vt_sb = work.tile([128, 512], bf16, tag='vt', bufs=2)
                nc.vector.tensor_copy(vt_sb[:], vt_ps[:])
                for c in range(4):
                    x8_t = work.tile([128, 2, 512], f8, tag='x8', bufs=4)
                    nc.sync.dma_start(out=x8_t[:],
                                      in_=xt8[c, :, :, ts:ts + 512])
                    st, sp = (c == 0), (c == 3)
                    nc.tensor.matmul(q_ps[:], wqk8_sb[c][:, :, 0:128],
                                     x8_t[:], start=st, stop=sp, perf_mode=DR)
                    nc.tensor.matmul(k_ps[:], wqk8_sb[c][:, :, 128:256],
                                     x8_t[:], start=st, stop=sp, perf_mode=DR)
                nc.vector.tensor_copy(qT_sb[tb][:], q_ps[:])
                nc.vector.tensor_copy(kT_sb[tb][:], k_ps[:])
                # transpose vT -> v [s, d] per 128-chunk (bf16 PE transpose)
                for sc in range(4):
                    si = tb * 4 + sc
                    vtr = ps.tile([128, 128], bf16, tag='misc', bufs=2)
                    nc.tensor.transpose(vtr[:], vt_sb[:, sc * 128:(sc + 1) * 128],
                                        identb[:])
                    nc.vector.tensor_copy(
                        v_sb[si][:, :, 0:64],
                        vtr.rearrange('p (h e) -> p h e', h=2))

            # ---- phase 2a: attention (scores+exp+AV) for one t-block ----
            def emit_attn_block(b, tb):
                n_si = 4 * (tb + 1)            # s-tiles (causal)
                av_pss = [ps.tile([65, 512], f32, tag='acc', bufs=4,
                                  name=f'av{b}_{tb}_{h}')
                          for h in range(HPC)]
                for si in range(n_si):
                    s0 = si * 128
                    sblk, srem = divmod(s0, 512)
                    sblk += b * NTB
                    woff = 0
                    if si >= 4 * tb:           # diagonal region
                        woff = (si - 4 * tb) * 128
                    # f32r matmuls run at 1/4 rate below N=256: clamp
                    moff = min(woff, 256)
                    for h in range(HPC):
                        hd = h * 64
                        w_ps = ps.tile([128, 512], f32, tag='wei', bufs=2)
                        nc.tensor.matmul(
                            w_ps[:, moff:512],
                            kT_sb[sblk][hd:hd + 64, srem:srem + 128],
                            qT_sb[b * NTB + tb][hd:hd + 64, moff:512],
                            start=True, stop=True)
                        wt = work.tile([128, 512], bf16, tag='weiT', bufs=24)
                        nc.scalar.activation(wt[:, woff:512], w_ps[:, woff:512],
                                             EXP, scale=EXP_SCALE)
                        if si >= 4 * tb:
                            nc.gpsimd.tensor_mul(wt[:, woff:woff + 128],
                                                 wt[:, woff:woff + 128],
                                                 tmask_b[:])
                        nc.tensor.matmul(
                            av_pss[h][:, woff:512],
                            v_sb[b * NST + si][:, h, :], wt[:, woff:512],
                            start=(si == 0), stop=(si == n_si - 1))
                return av_pss

            # ---- phase 2b: normalize + projection for one t-block ----
            def emit_norm_proj(b, tb, av_pss):
                t0 = b * T + tb * 512
                lrow = work.tile([33, 512], f32, tag='lrow', bufs=2)
                nc.gpsimd.memset(lrow[:], 1.0)
                for h in range(HPC):
                    nc.vector.tensor_copy(lrow[32 * h:32 * h + 1, :],
                                          av_pss[h][64:65, :])
                rc2 = work.tile([33, 512], f32, tag='rc', bufs=2)
                nc.vector.reciprocal(rc2[:], lrow[:])
                rc2r = work.tile([33, 512], f32r, tag='rcr', bufs=2)
                nc.vector.tensor_copy(rc2r[:], rc2[:])
                bc_ps = ps.tile([128, 512], f32, tag='misc', bufs=2)
                nc.tensor.matmul(bc_ps[:], sel_bc[:], rc2r[:],
                                 start=True, stop=True)
                bc_sb = work.tile([128, 512], f32, tag='bcs', bufs=2)
                nc.vector.tensor_copy(bc_sb[:], bc_ps[:])
                avT_sb = work.tile([128, 512], f32r, tag='avT', bufs=2)
                for h in range(HPC):
                    hd = h * 64
                    nc.vector.tensor_mul(avT_sb[hd:hd + 64, :],
                                         av_pss[h][0:64, :],
                                         bc_sb[hd:hd + 64, :])
                # ---- projection for this 512-t-block ----
                for tc4 in range(4):
                    for eb in range(2):
                        y_ps = ps.tile([128, 512], f32, tag='misc', bufs=2)
                        nc.tensor.matmul(
                            y_ps[:],
                            avT_sb[:, tc4 * 128:(tc4 + 1) * 128],
                            wproj_sb[:, eb * 512:(eb + 1) * 512],
                            start=True, stop=True)
                        y_sb = work.tile([128, 512], bf16, tag='ysb', bufs=6)
                        if (tc4 * 2 + eb) % 2 == 0:
                            nc.scalar.activation(y_sb[:], y_ps[:], CPY)
                        else:
                            nc.vector.tensor_copy(y_sb[:], y_ps[:])
                        nc.sync.dma_start(
                            out=y[t0 + tc4 * 128:t0 + (tc4 + 1) * 128,
                                  eb * 512:(eb + 1) * 512],
                            in_=y_sb[:])

            # ---- schedule: pipeline normalize/proj one block behind.
            # np(0,3) must land right after qkv(4): the b1 QKV q/k PSUM
            # tiles share the 'acc' ring with b0's last av tiles, and their
            # DVE casts precede np DVE ops in engine order — emitting all
            # four qkv blocks before np(0,3) would deadlock DVE.
            for j in range(4):
                emit_qkv_block(j)
            avs = {}
            for tb in range(NTB):
                avs[(0, tb)] = emit_attn_block(0, tb)
                if tb > 0:
                    emit_norm_proj(0, tb - 1, avs.pop((0, tb - 1)))
            emit_qkv_block(4)
            emit_norm_proj(0, NTB - 1, avs.pop((0, NTB - 1)))
            for j in range(5, 8):
                emit_qkv_block(j)
            for tb in range(NTB):
                avs[(1, tb)] = emit_attn_block(1, tb)
                if tb > 0:
                    emit_norm_proj(1, tb - 1, avs.pop((1, tb - 1)))
            emit_norm_proj(1, NTB - 1, avs.pop((1, NTB - 1)))

    import concourse.mybir as mybir2
    _split_multi_waits(nc, mybir2)
    return nc


_CACHE = {}


def kernel(x, Wq, Wk, Wv, Wproj, bproj):
    _install_ntff_hook()
    from concourse.bass_utils import run_bass_kernel_spmd

    x = np.asarray(x, dtype=np.float32)
    Wq = np.asarray(Wq, dtype=np.float32)
    Wk = np.asarray(Wk, dtype=np.float32)
    Wv = np.asarray(Wv, dtype=np.float32)
    Wproj = np.asarray(Wproj, dtype=np.float32)
    bproj = np.asarray(bproj, dtype=np.float32)

    if 'nc' not in _CACHE:
        _CACHE['nc'] = _build_nc()
    nc = _CACHE['nc']

    e4m3 = ml_dtypes.float8_e4m3fn
    bf = ml_dtypes.bfloat16
    xT = np.ascontiguousarray(x.reshape(BT, E).T)          # [E, BT]
    # fp8 pair layout: e = c*256 + i*128 + p  ->  xt8[c, p, i, t]
    xt8 = np.ascontiguousarray(
        xT.reshape(4, 2, 128, BT).transpose(0, 2, 1, 3)).astype(e4m3)
    xtb = xT.astype(bf)

    in_maps = []
    for c in range(N_CORES):
        h0 = HPC * c
        wq_c = np.concatenate([Wq[h0], Wq[h0 + 1]], axis=1)    # [E, 128]
        wk_c = np.concatenate([Wk[h0], Wk[h0 + 1]], axis=1)
        wqk = np.concatenate([wq_c, wk_c], axis=1) * WSCALE    # [E, 256]
        wqk8_c = np.ascontiguousarray(
            wqk.reshape(4, 2, 128, 256).transpose(0, 2, 1, 3)).astype(e4m3)
        wv_c = np.concatenate([Wv[h0], Wv[h0 + 1]], axis=1).astype(bf)
        wproj_c = np.ascontiguousarray(Wproj[DPC * c: DPC * (c + 1)])
        in_maps.append({'xt8': xt8, 'xtb': xtb, 'wqk8': wqk8_c,
                        'wvb': np.ascontiguousarray(wv_c),
                        'wproj': wproj_c})

    res = run_bass_kernel_spmd(nc, in_maps, list(range(N_CORES)))
    ysum = np.zeros((BT, E), dtype=np.float64)
    for c in range(N_CORES):
        ysum += np.asarray(res.results[c]['y']).astype(np.float64)
    out = (ysum + bproj.astype(np.float64)).astype(np.float32)
    return out.reshape(B, T, E)
